# revision 27
# baseline (speedup 1.0000x reference)
"""Ball-query KNN (radius=0.25, k=10) for Q=16384 queries over N=16384 points.

Strategy (8 NeuronCores, queries sharded 2048/core, spatially pruned):
  - Host: bucket points on a 32^3 grid; per query, find a PROVABLY safe
    upper bound b_q on its 10th-NN distance (smallest cell ring whose cube
    holds >= 10 points => b_q = (k+1)*h*sqrt(3)).  Sort queries by cell so
    each 128-query block is spatially tight; the block's candidate list is
    every point within the block bbox expanded by max b_q.  All blocks are
    padded to one fixed NPB with far-away sentinel points.
  - PE: augmented K=5 fp32 matmul computes d2 = q2 + p2 - 2 q.p per block
    over its NPB candidates (lhsT rows per query: [q2, 1, -2qx, -2qy,
    -2qz]; rhs rows per candidate point: [1, p2, px, py, pz]).
  - ACT: PSUM -> SBUF copy with scale=-1 => s = -d2.
  - DVE: exact fp32 top-16 of each row via max8 + max_index +
    match_replace + max8 + max_index (order and ties exactly as the
    value-sorted reference).
  - Host: local->global index decode, validity (s >= -r^2), mapping
    zeroing, coords gather, inverse query permutation, unshard.
The candidate sets provably contain each query's true top-10, so the
result is identical to brute force over all 16384 points.
"""

import numpy as np

N_CORES = 8
N = 16384          # points
Q = 16384          # queries (flattened 32*32*16)
QL = Q // N_CORES  # queries per core
P = 128            # partitions / queries per block
NBLK = QL // P     # 16 blocks per core
GBLK = Q // P      # 128 global blocks
K = 5              # augmented contraction dim
MM = 512           # matmul free dim (1 bank fp32)
CH = 2048          # psum chunk (4 banks)
RADIUS2 = 0.0625   # 0.25**2
NEG_INF = -3.0e38
G = 32             # host pruning grid
SENT = 4.0         # sentinel coordinate (far outside the unit cube)

_CACHE = {}


def _build_bass(npb):
    import concourse.bacc as bacc
    import concourse.mybir as mybir
    from concourse.tile import TileContext

    f32 = mybir.dt.float32
    u32 = mybir.dt.uint32

    nc = bacc.Bacc("TRN2", target_bir_lowering=False, debug=False,
                   enable_asserts=False)
    # per-block candidate points (augmented, NBLK*npb) then the core's
    # queries (augmented) in ONE tensor: a single DMA = single semaphore, so
    # the first matmul carries one sync wait (walrus limit on LDWEIGHTS).
    pq_d = nc.dram_tensor("pq", [K, NBLK * npb + QL], f32,
                          kind="ExternalInput").ap()
    out_d = nc.dram_tensor("out_iv", [QL, 32], f32, kind="ExternalOutput").ap()

    n_ch = (npb + CH - 1) // CH

    with TileContext(nc) as tc:
        with (
            tc.tile_pool(name="consts", bufs=1) as constp,
            tc.tile_pool(name="pablk", bufs=3) as pablkp,
            tc.tile_pool(name="ps", bufs=2, space="PSUM") as psp,
            tc.tile_pool(name="sblk", bufs=2) as sblkp,
            tc.tile_pool(name="small", bufs=4) as smallp,
            tc.tile_pool(name="outb", bufs=3) as outbp,
        ):
            qa_all = constp.tile([K, QL], f32, tag="qa")
            nc.sync.dma_start(out=qa_all, in_=pq_d[:, NBLK * npb:])

            for b in range(NBLK):
                qa_sb = qa_all[:, b * P:(b + 1) * P]
                pa_sb = pablkp.tile([K, npb], f32, tag="pablk")
                nc.sync.dma_start(out=pa_sb,
                                  in_=pq_d[:, b * npb:(b + 1) * npb])

                s_blk = sblkp.tile([P, npb], f32, tag="s_blk")
                for c in range(n_ch):
                    ch = min(CH, npb - c * CH)
                    ps = psp.tile([P, CH], f32, tag="ps")
                    for m in range(0, ch, MM):
                        mm = min(MM, ch - m)
                        nc.tensor.matmul(
                            ps[:, m:m + mm],
                            lhsT=qa_sb,
                            rhs=pa_sb[:, c * CH + m:c * CH + m + mm],
                            start=True, stop=True,
                        )
                        nc.scalar.activation(
                            s_blk[:, c * CH + m:c * CH + m + mm],
                            ps[:, m:m + mm],
                            mybir.ActivationFunctionType.Copy,
                            scale=-1.0,
                        )

                out_sb = outbp.tile([P, 32], f32, tag="out_sb")

                # exact fp32 top-16 (values + local indices), rank order
                t8v = smallp.tile([P, 8], f32, tag="t8v")
                nc.vector.max(out=t8v, in_=s_blk)
                t8p = smallp.tile([P, 8], u32, tag="t8p")
                nc.vector.max_index(out=t8p, in_max=t8v, in_values=s_blk)

                s_blk2 = sblkp.tile([P, npb], f32, tag="s_blk2")
                nc.vector.match_replace(out=s_blk2, in_to_replace=t8v,
                                        in_values=s_blk, imm_value=NEG_INF)
                n8v = smallp.tile([P, 8], f32, tag="n8v")
                nc.vector.max(out=n8v, in_=s_blk2)
                n8p = smallp.tile([P, 8], u32, tag="n8p")
                nc.vector.max_index(out=n8p, in_max=n8v, in_values=s_blk2)

                nc.vector.tensor_copy(out_sb[:, 0:8], t8p)
                nc.vector.tensor_copy(out_sb[:, 8:16], n8p)
                nc.vector.tensor_copy(out_sb[:, 16:24], t8v)
                nc.vector.tensor_copy(out_sb[:, 24:32], n8v)

                nc.sync.dma_start(out=out_d[b * P:(b + 1) * P, :], in_=out_sb)

    nc.finalize()
    return nc


def _host_prune(xp, qp):
    """Spatially sort queries and build per-block candidate lists.

    Returns (order, blk_gids [GBLK, npb] int32, npb).  Candidate list of
    each block provably contains every member query's true 10 nearest
    neighbors (by the ring-count bound).
    """
    h = 1.0 / G
    pc = np.clip((xp * G).astype(np.int64), 0, G - 1)       # (N,3)
    qc = np.clip((qp * G).astype(np.int64), 0, G - 1)       # (Q,3)
    # 3D cell counts and inclusive prefix sums for O(1) cube counts
    cnt = np.zeros((G, G, G), np.int64)
    np.add.at(cnt, (pc[:, 0], pc[:, 1], pc[:, 2]), 1)
    S = np.zeros((G + 1, G + 1, G + 1), np.int64)
    S[1:, 1:, 1:] = cnt.cumsum(0).cumsum(1).cumsum(2)

    def cube_count(c, k):
        lo = np.clip(c - k, 0, G)
        hi = np.clip(c + k + 1, 0, G)
        return (S[hi[:, 0], hi[:, 1], hi[:, 2]]
                - S[lo[:, 0], hi[:, 1], hi[:, 2]]
                - S[hi[:, 0], lo[:, 1], hi[:, 2]]
                - S[hi[:, 0], hi[:, 1], lo[:, 2]]
                + S[lo[:, 0], lo[:, 1], hi[:, 2]]
                + S[lo[:, 0], hi[:, 1], lo[:, 2]]
                + S[hi[:, 0], lo[:, 1], lo[:, 2]]
                - S[lo[:, 0], lo[:, 1], lo[:, 2]])

    b_q = np.full(Q, -1.0)
    for k in range(1, 8):
        need = b_q < 0
        if not need.any():
            break
        cc = cube_count(qc[need], k)
        ok = cc >= 10
        idxs = np.where(need)[0][ok]
        b_q[idxs] = (k + 1) * h * np.sqrt(3.0)
    assert (b_q > 0).all(), "ring bound not found for some query"

    # sort queries by cell id -> spatially tight blocks
    qcell = (qc[:, 0] * G + qc[:, 1]) * G + qc[:, 2]
    order = np.argsort(qcell, kind="stable")

    blk_gids = []
    npb = 0
    for gb in range(GBLK):
        qs = order[gb * P:(gb + 1) * P]
        bq = b_q[qs][:, None]
        lo = (qp[qs] - bq).min(axis=0)
        hi = (qp[qs] + bq).max(axis=0)
        m = ((xp >= lo) & (xp <= hi)).all(axis=1)
        gids = np.where(m)[0].astype(np.int32)
        blk_gids.append(gids)
        npb = max(npb, len(gids))
    npb = ((npb + 127) // 128) * 128
    gid_arr = np.full((GBLK, npb), -1, np.int32)
    for gb, g in enumerate(blk_gids):
        gid_arr[gb, :len(g)] = g
    return order, gid_arr, npb


def _prep_aug(xp, qp):
    f = np.float32
    px, py, pz = xp[:, 0], xp[:, 1], xp[:, 2]
    p2 = (px * px + py * py) + pz * pz
    pa = np.stack([np.ones(len(xp), f), p2, px, py, pz]).astype(f)
    qx, qy, qz = qp[:, 0], qp[:, 1], qp[:, 2]
    q2 = (qx * qx + qy * qy) + qz * qz
    qa = np.stack([q2, np.ones(len(qp), f), -2 * qx, -2 * qy, -2 * qz])
    return pa, qa.astype(f)


def kernel(x, p_grid):
    from concourse.bass_utils import run_bass_kernel_spmd

    f = np.float32
    xp = np.asarray(x, f).reshape(N, 3)
    qp = np.asarray(p_grid, f).reshape(Q, 3)

    order, gid_arr, npb = _host_prune(xp, qp)
    qp_s = qp[order]
    pa, qa = _prep_aug(xp, qp_s)

    # per-block augmented candidate rows, sentinel-padded
    sent = np.array([1.0, 3 * SENT * SENT, SENT, SENT, SENT], f)
    pa_blocks = np.empty((GBLK, K, npb), f)
    pa_blocks[:] = sent[None, :, None]
    for gb in range(GBLK):
        g = gid_arr[gb]
        val = g >= 0
        pa_blocks[gb, :, :val.sum()] = pa[:, g[val]]

    if ("nc", npb) not in _CACHE:
        _CACHE[("nc", npb)] = _build_bass(npb)
    nc = _CACHE[("nc", npb)]

    in_maps = []
    for c in range(N_CORES):
        blocks = pa_blocks[c * NBLK:(c + 1) * NBLK]       # (NBLK, K, npb)
        pq = np.concatenate(
            [np.concatenate(list(blocks), axis=1),
             qa[:, c * QL:(c + 1) * QL]], axis=1)
        in_maps.append({"pq": np.ascontiguousarray(pq)})

    res = run_bass_kernel_spmd(nc, in_maps, core_ids=list(range(N_CORES)),
                               **_CACHE.get("run_kwargs", {}))
    _CACHE["last_results"] = res
    iv = np.stack([res.results[c]["out_iv"] for c in range(N_CORES)])
    iv = iv.reshape(Q, 32)
    lidx = iv[:, :10].astype(np.int64)                    # local idx, rank order
    vals = iv[:, 16:26]

    # local -> global ids via per-block tables
    blk_of_q = np.repeat(np.arange(GBLK), P)
    gidx = gid_arr[blk_of_q[:, None], lidx]
    valid = vals >= np.float32(-RADIUS2)
    mapping_s = np.where(valid, gidx, 0).astype(np.int32)
    outputs_s = np.where(valid[..., None], xp[mapping_s], np.float32(0.0))

    # inverse permutation back to original query order
    inv = np.empty(Q, np.int64)
    inv[order] = np.arange(Q)
    mapping = mapping_s[inv]
    outputs = outputs_s[inv]
    return mapping[None], outputs[None].astype(np.float32)


# revision 31
# speedup vs baseline: 2.3535x; 2.3535x over previous
"""Ball-query KNN (radius=0.25, k=10) for Q=16384 queries over N=16384 points.

Strategy (8 NeuronCores, queries sharded 2048/core, spatially pruned):
  - Host: bucket points on a 32^3 grid; per query, find a PROVABLY safe
    upper bound b_q on its 10th-NN distance (smallest cell ring whose cube
    holds >= 10 points => b_q = (k+1)*h*sqrt(3)).  Sort queries by cell so
    each 128-query block is spatially tight; the block's candidate list is
    every point within the block bbox expanded per-query by b_q.  Blocks are
    dealt round-robin by descending candidate count so block-slot j has the
    same width on every core (one SPMD NEFF, per-slot widths, sentinel pad).
  - PE: augmented K=5 fp32 matmul computes d2 = q2 + p2 - 2 q.p per block
    over its NPB candidates (lhsT rows per query: [q2, 1, -2qx, -2qy,
    -2qz]; rhs rows per candidate point: [1, p2, px, py, pz]).
  - ACT: PSUM -> SBUF copy with scale=-1 => s = -d2.
  - DVE: exact fp32 top-16 of each row via max8 + max_index +
    match_replace + max8 + max_index (order and ties exactly as the
    value-sorted reference).
  - Host: local->global index decode, validity (s >= -r^2), mapping
    zeroing, coords gather, inverse query permutation, unshard.
The candidate sets provably contain each query's true top-10, so the
result is identical to brute force over all 16384 points.
"""

import numpy as np

N_CORES = 8
N = 16384          # points
Q = 16384          # queries (flattened 32*32*16)
QL = Q // N_CORES  # queries per core
P = 128            # partitions / queries per block
NBLK = QL // P     # 16 blocks per core
GBLK = Q // P      # 128 global blocks
K = 5              # augmented contraction dim
MM = 512           # matmul free dim (1 bank fp32)
CH = 2048          # psum chunk (4 banks)
RADIUS2 = 0.0625   # 0.25**2
NEG_INF = -3.0e38
G = 32             # host pruning grid
SENT = 4.0         # sentinel coordinate (far outside the unit cube)

_CACHE = {}


def _build_bass(npbs):
    import concourse.bacc as bacc
    import concourse.mybir as mybir
    from concourse.tile import TileContext

    f32 = mybir.dt.float32
    u32 = mybir.dt.uint32

    nc = bacc.Bacc("TRN2", target_bir_lowering=False, debug=False,
                   enable_asserts=False)
    # per-slot candidate points (augmented, variable widths) then the core's
    # queries (augmented) in ONE tensor: a single DMA = single semaphore, so
    # the first matmul carries one sync wait (walrus limit on LDWEIGHTS).
    tot = sum(npbs)
    offs = np.concatenate([[0], np.cumsum(npbs)]).tolist()
    pq_d = nc.dram_tensor("pq", [K, tot + QL], f32,
                          kind="ExternalInput").ap()
    out_d = nc.dram_tensor("out_iv", [QL, 32], f32, kind="ExternalOutput").ap()

    with TileContext(nc) as tc:
        with (
            tc.tile_pool(name="consts", bufs=1) as constp,
            tc.tile_pool(name="pablk", bufs=3) as pablkp,
            tc.tile_pool(name="ps", bufs=2, space="PSUM") as psp,
            tc.tile_pool(name="sblk", bufs=2) as sblkp,
            tc.tile_pool(name="small", bufs=4) as smallp,
            tc.tile_pool(name="outb", bufs=3) as outbp,
        ):
            qa_all = constp.tile([K, QL], f32, tag="qa")
            nc.sync.dma_start(out=qa_all, in_=pq_d[:, tot:])

            for b in range(NBLK):
                npb = npbs[b]
                qa_sb = qa_all[:, b * P:(b + 1) * P]
                pa_sb = pablkp.tile([K, npb], f32, tag="pablk")
                nc.sync.dma_start(out=pa_sb,
                                  in_=pq_d[:, offs[b]:offs[b + 1]])

                s_blk = sblkp.tile([P, npb], f32, tag="s_blk")
                for c in range(0, npb, CH):
                    ch = min(CH, npb - c)
                    ps = psp.tile([P, CH], f32, tag="ps")
                    for m in range(0, ch, MM):
                        mm = min(MM, ch - m)
                        nc.tensor.matmul(
                            ps[:, m:m + mm],
                            lhsT=qa_sb,
                            rhs=pa_sb[:, c + m:c + m + mm],
                            start=True, stop=True,
                        )
                        nc.scalar.activation(
                            s_blk[:, c + m:c + m + mm],
                            ps[:, m:m + mm],
                            mybir.ActivationFunctionType.Copy,
                            scale=-1.0,
                        )

                out_sb = outbp.tile([P, 32], f32, tag="out_sb")

                # exact fp32 top-16 (values + local indices), rank order
                t8v = smallp.tile([P, 8], f32, tag="t8v")
                nc.vector.max(out=t8v, in_=s_blk)
                t8p = smallp.tile([P, 8], u32, tag="t8p")
                nc.vector.max_index(out=t8p, in_max=t8v, in_values=s_blk)

                s_blk2 = sblkp.tile([P, npb], f32, tag="s_blk2")
                nc.vector.match_replace(out=s_blk2, in_to_replace=t8v,
                                        in_values=s_blk, imm_value=NEG_INF)
                n8v = smallp.tile([P, 8], f32, tag="n8v")
                nc.vector.max(out=n8v, in_=s_blk2)
                n8p = smallp.tile([P, 8], u32, tag="n8p")
                nc.vector.max_index(out=n8p, in_max=n8v, in_values=s_blk2)

                nc.vector.tensor_copy(out_sb[:, 0:8], t8p)
                nc.vector.tensor_copy(out_sb[:, 8:16], n8p)
                nc.vector.tensor_copy(out_sb[:, 16:24], t8v)
                nc.vector.tensor_copy(out_sb[:, 24:32], n8v)

                nc.sync.dma_start(out=out_d[b * P:(b + 1) * P, :], in_=out_sb)

    nc.finalize()
    return nc


def _host_prune(xp, qp):
    """Spatially sort queries and build per-block candidate lists.

    Returns (order, blk_gids [GBLK, npb] int32, npb).  Candidate list of
    each block provably contains every member query's true 10 nearest
    neighbors (by the ring-count bound).
    """
    h = 1.0 / G
    pc = np.clip((xp * G).astype(np.int64), 0, G - 1)       # (N,3)
    qc = np.clip((qp * G).astype(np.int64), 0, G - 1)       # (Q,3)
    # 3D cell counts and inclusive prefix sums for O(1) cube counts
    cnt = np.zeros((G, G, G), np.int64)
    np.add.at(cnt, (pc[:, 0], pc[:, 1], pc[:, 2]), 1)
    S = np.zeros((G + 1, G + 1, G + 1), np.int64)
    S[1:, 1:, 1:] = cnt.cumsum(0).cumsum(1).cumsum(2)

    def cube_count(c, k):
        lo = np.clip(c - k, 0, G)
        hi = np.clip(c + k + 1, 0, G)
        return (S[hi[:, 0], hi[:, 1], hi[:, 2]]
                - S[lo[:, 0], hi[:, 1], hi[:, 2]]
                - S[hi[:, 0], lo[:, 1], hi[:, 2]]
                - S[hi[:, 0], hi[:, 1], lo[:, 2]]
                + S[lo[:, 0], lo[:, 1], hi[:, 2]]
                + S[lo[:, 0], hi[:, 1], lo[:, 2]]
                + S[hi[:, 0], lo[:, 1], lo[:, 2]]
                - S[lo[:, 0], lo[:, 1], lo[:, 2]])

    b_q = np.full(Q, -1.0)
    for k in range(1, 8):
        need = b_q < 0
        if not need.any():
            break
        cc = cube_count(qc[need], k)
        ok = cc >= 10
        idxs = np.where(need)[0][ok]
        b_q[idxs] = (k + 1) * h * np.sqrt(3.0)
    assert (b_q > 0).all(), "ring bound not found for some query"

    # sort queries by cell id -> spatially tight blocks
    qcell = (qc[:, 0] * G + qc[:, 1]) * G + qc[:, 2]
    order = np.argsort(qcell, kind="stable")

    blk_gids = []
    for gb in range(GBLK):
        qs = order[gb * P:(gb + 1) * P]
        bq = b_q[qs][:, None]
        lo = (qp[qs] - bq).min(axis=0)
        hi = (qp[qs] + bq).max(axis=0)
        m = ((xp >= lo) & (xp <= hi)).all(axis=1)
        blk_gids.append(np.where(m)[0].astype(np.int32))
    return order, blk_gids


def _prep_aug(xp, qp):
    f = np.float32
    px, py, pz = xp[:, 0], xp[:, 1], xp[:, 2]
    p2 = (px * px + py * py) + pz * pz
    pa = np.stack([np.ones(len(xp), f), p2, px, py, pz]).astype(f)
    qx, qy, qz = qp[:, 0], qp[:, 1], qp[:, 2]
    q2 = (qx * qx + qy * qy) + qz * qz
    qa = np.stack([q2, np.ones(len(qp), f), -2 * qx, -2 * qy, -2 * qz])
    return pa, qa.astype(f)


def kernel(x, p_grid):
    from concourse.bass_utils import run_bass_kernel_spmd

    f = np.float32
    xp = np.asarray(x, f).reshape(N, 3)
    qp = np.asarray(p_grid, f).reshape(Q, 3)

    order, blk_gids = _host_prune(xp, qp)
    pa, qa_full = _prep_aug(xp, qp)

    # Deal blocks round-robin by descending candidate count: slot j of core c
    # gets global block rank[j*8+c], so slot j has the same (max-of-8) width
    # on every core and the one NEFF serves all cores.
    counts = np.array([len(g) for g in blk_gids])
    rank = np.argsort(-counts, kind="stable")
    slot_blk = rank.reshape(NBLK, N_CORES)                 # [slot, core]
    npbs = [int(((counts[slot_blk[j]].max() + 127) // 128) * 128)
            for j in range(NBLK)]

    if ("nc", tuple(npbs)) not in _CACHE:
        _CACHE[("nc", tuple(npbs))] = _build_bass(npbs)
    nc = _CACHE[("nc", tuple(npbs))]

    sent = np.array([1.0, 3 * SENT * SENT, SENT, SENT, SENT], f)
    in_maps = []
    final_order = np.empty(Q, np.int64)
    for c in range(N_CORES):
        parts = []
        qidx = []
        for j in range(NBLK):
            g = blk_gids[slot_blk[j, c]]
            pad = np.empty((K, npbs[j]), f)
            pad[:] = sent[:, None]
            pad[:, :len(g)] = pa[:, g]
            parts.append(pad)
            qidx.append(order[slot_blk[j, c] * P:(slot_blk[j, c] + 1) * P])
        qidx = np.concatenate(qidx)
        final_order[c * QL:(c + 1) * QL] = qidx
        parts.append(qa_full[:, qidx])
        in_maps.append({"pq": np.ascontiguousarray(
            np.concatenate(parts, axis=1))})

    res = run_bass_kernel_spmd(nc, in_maps, core_ids=list(range(N_CORES)),
                               **_CACHE.get("run_kwargs", {}))
    _CACHE["last_results"] = res
    iv = np.stack([res.results[c]["out_iv"] for c in range(N_CORES)])
    iv = iv.reshape(Q, 32)
    lidx = iv[:, :10].astype(np.int64)                    # local idx, rank order
    vals = iv[:, 16:26]

    # local -> global ids via per-slot tables (padded to the widest slot)
    width = max(npbs)
    gid_tab = np.zeros((Q // P, width), np.int32)
    row = 0
    for c in range(N_CORES):
        for j in range(NBLK):
            g = blk_gids[slot_blk[j, c]]
            gid_tab[row, :len(g)] = g
            row += 1
    blk_of_q = np.repeat(np.arange(Q // P), P)
    gidx = gid_tab[blk_of_q[:, None], lidx]
    valid = vals >= np.float32(-RADIUS2)
    mapping_s = np.where(valid, gidx, 0).astype(np.int32)
    outputs_s = np.where(valid[..., None], xp[mapping_s], np.float32(0.0))

    # inverse permutation back to original query order
    inv = np.empty(Q, np.int64)
    inv[final_order] = np.arange(Q)
    mapping = mapping_s[inv]
    outputs = outputs_s[inv]
    return mapping[None], outputs[None].astype(np.float32)


# revision 34
# speedup vs baseline: 3.6749x; 1.5615x over previous
"""Ball-query KNN (radius=0.25, k=10) for Q=16384 queries over N=16384 points.

Strategy (8 NeuronCores, queries sharded 2048/core, spatially pruned):
  - Host: bucket points on a 32^3 grid; per query, find a PROVABLY safe
    upper bound b_q on its 10th-NN distance (smallest cell ring whose cube
    holds >= 10 points => b_q = (k+1)*h*sqrt(3)).  Sort queries by cell so
    each 128-query block is spatially tight; the block's candidate list is
    every point within the block bbox expanded per-query by b_q.  Blocks are
    dealt round-robin by descending candidate count so block-slot j has the
    same width on every core (one SPMD NEFF, per-slot widths, sentinel pad).
  - PE: augmented K=5 fp32 matmul computes d2 = q2 + p2 - 2 q.p per block
    over its NPB candidates (lhsT rows per query: [q2, 1, -2qx, -2qy,
    -2qz]; rhs rows per candidate point: [1, p2, px, py, pz]).
  - ACT: PSUM -> SBUF copy with scale=-1 => s = -d2.
  - DVE: exact fp32 top-16 of each row via max8 + max_index +
    match_replace + max8 + max_index (order and ties exactly as the
    value-sorted reference).
  - Host: local->global index decode, validity (s >= -r^2), mapping
    zeroing, coords gather, inverse query permutation, unshard.
The candidate sets provably contain each query's true top-10, so the
result is identical to brute force over all 16384 points.
"""

import numpy as np

N_CORES = 8
N = 16384          # points
Q = 16384          # queries (flattened 32*32*16)
QL = Q // N_CORES  # queries per core
P = 128            # partitions / queries per block
NBLK = QL // P     # 16 blocks per core
GBLK = Q // P      # 128 global blocks
K = 5              # augmented contraction dim
MM = 512           # matmul free dim (1 bank fp32)
CH = 2048          # psum chunk (4 banks)
RADIUS2 = 0.0625   # 0.25**2
NEG_INF = -3.0e38
G = 32             # host pruning grid
SENT = 4.0         # sentinel coordinate (far outside the unit cube)

_CACHE = {}


def _build_bass(npbs):
    import concourse.bacc as bacc
    import concourse.mybir as mybir
    from concourse.tile import TileContext

    f32 = mybir.dt.float32
    u32 = mybir.dt.uint32

    nc = bacc.Bacc("TRN2", target_bir_lowering=False, debug=False,
                   enable_asserts=False)
    # per-slot candidate points (augmented, variable widths) then the core's
    # queries (augmented) in ONE tensor: a single DMA = single semaphore, so
    # the first matmul carries one sync wait (walrus limit on LDWEIGHTS).
    tot = sum(npbs)
    offs = np.concatenate([[0], np.cumsum(npbs)]).tolist()
    pq_d = nc.dram_tensor("pq", [K, tot + QL], f32,
                          kind="ExternalInput").ap()
    out_d = nc.dram_tensor("out_iv", [QL, 32], f32, kind="ExternalOutput").ap()

    with TileContext(nc) as tc:
        with (
            tc.tile_pool(name="consts", bufs=1) as constp,
            tc.tile_pool(name="pablk", bufs=4) as pablkp,
            tc.tile_pool(name="ps", bufs=2, space="PSUM") as psp,
            tc.tile_pool(name="sblk", bufs=3) as sblkp,
            tc.tile_pool(name="small", bufs=6) as smallp,
            tc.tile_pool(name="outb", bufs=3) as outbp,
        ):
            qa_all = constp.tile([K, QL], f32, tag="qa")
            nc.sync.dma_start(out=qa_all, in_=pq_d[:, tot:])

            for b in range(NBLK):
                npb = npbs[b]
                qa_sb = qa_all[:, b * P:(b + 1) * P]
                pa_sb = pablkp.tile([K, npb], f32, tag="pablk")
                nc.sync.dma_start(out=pa_sb,
                                  in_=pq_d[:, offs[b]:offs[b + 1]])

                s_blk = sblkp.tile([P, npb], f32, tag="s_blk")
                for c in range(0, npb, CH):
                    ch = min(CH, npb - c)
                    ps = psp.tile([P, CH], f32, tag="ps")
                    for m in range(0, ch, MM):
                        mm = min(MM, ch - m)
                        nc.tensor.matmul(
                            ps[:, m:m + mm],
                            lhsT=qa_sb,
                            rhs=pa_sb[:, c + m:c + m + mm],
                            start=True, stop=True,
                        )
                        nc.scalar.activation(
                            s_blk[:, c + m:c + m + mm],
                            ps[:, m:m + mm],
                            mybir.ActivationFunctionType.Copy,
                            scale=-1.0,
                        )

                out_sb = outbp.tile([P, 32], f32, tag="out_sb")

                # exact fp32 top-16 (values + local indices), rank order
                t8v = smallp.tile([P, 8], f32, tag="t8v")
                nc.vector.max(out=t8v, in_=s_blk)
                t8p = smallp.tile([P, 8], u32, tag="t8p")
                nc.vector.max_index(out=t8p, in_max=t8v, in_values=s_blk)

                s_blk2 = sblkp.tile([P, npb], f32, tag="s_blk2")
                nc.vector.match_replace(out=s_blk2, in_to_replace=t8v,
                                        in_values=s_blk, imm_value=NEG_INF)
                n8v = smallp.tile([P, 8], f32, tag="n8v")
                nc.vector.max(out=n8v, in_=s_blk2)
                n8p = smallp.tile([P, 8], u32, tag="n8p")
                nc.vector.max_index(out=n8p, in_max=n8v, in_values=s_blk2)

                nc.vector.tensor_copy(out_sb[:, 0:8], t8p)
                nc.vector.tensor_copy(out_sb[:, 8:16], n8p)
                nc.vector.tensor_copy(out_sb[:, 16:24], t8v)
                nc.vector.tensor_copy(out_sb[:, 24:32], n8v)

                nc.sync.dma_start(out=out_d[b * P:(b + 1) * P, :], in_=out_sb)

    nc.finalize()
    return nc


def _host_prune(xp, qp):
    """Spatially sort queries and build per-block candidate lists.

    Returns (order, blk_gids [GBLK, npb] int32, npb).  Candidate list of
    each block provably contains every member query's true 10 nearest
    neighbors (by the ring-count bound).
    """
    h = 1.0 / G
    pc = np.clip((xp * G).astype(np.int64), 0, G - 1)       # (N,3)
    qc = np.clip((qp * G).astype(np.int64), 0, G - 1)       # (Q,3)
    # 3D cell counts and inclusive prefix sums for O(1) cube counts
    cnt = np.zeros((G, G, G), np.int64)
    np.add.at(cnt, (pc[:, 0], pc[:, 1], pc[:, 2]), 1)
    S = np.zeros((G + 1, G + 1, G + 1), np.int64)
    S[1:, 1:, 1:] = cnt.cumsum(0).cumsum(1).cumsum(2)

    def cube_count(c, k):
        lo = np.clip(c - k, 0, G)
        hi = np.clip(c + k + 1, 0, G)
        return (S[hi[:, 0], hi[:, 1], hi[:, 2]]
                - S[lo[:, 0], hi[:, 1], hi[:, 2]]
                - S[hi[:, 0], lo[:, 1], hi[:, 2]]
                - S[hi[:, 0], hi[:, 1], lo[:, 2]]
                + S[lo[:, 0], lo[:, 1], hi[:, 2]]
                + S[lo[:, 0], hi[:, 1], lo[:, 2]]
                + S[hi[:, 0], lo[:, 1], lo[:, 2]]
                - S[lo[:, 0], lo[:, 1], lo[:, 2]])

    b_q = np.full(Q, -1.0)
    for k in range(1, 8):
        need = b_q < 0
        if not need.any():
            break
        cc = cube_count(qc[need], k)
        ok = cc >= 10
        idxs = np.where(need)[0][ok]
        b_q[idxs] = (k + 1) * h * np.sqrt(3.0)
    assert (b_q > 0).all(), "ring bound not found for some query"

    # tighten: 10th-smallest exact distance to a capped local subset is a
    # valid upper bound on the 10th-NN distance (any subset works).  Gather
    # up to CAP points per cell from the 27-cell cube on a coarse grid.
    G2 = 16
    CAP = 8
    pc2 = np.clip((xp * G2).astype(np.int64), 0, G2 - 1)
    qc2 = np.clip((qp * G2).astype(np.int64), 0, G2 - 1)
    cell2 = (pc2[:, 0] * G2 + pc2[:, 1]) * G2 + pc2[:, 2]
    sortp = np.argsort(cell2, kind="stable")
    cs = cell2[sortp]
    start = np.searchsorted(cs, np.arange(G2 ** 3))
    end = np.searchsorted(cs, np.arange(G2 ** 3), side="right")
    cols = []
    for dx in (-1, 0, 1):
        for dy in (-1, 0, 1):
            for dz in (-1, 0, 1):
                nb = qc2 + np.array([dx, dy, dz])
                inb = ((nb >= 0) & (nb < G2)).all(axis=1)
                nbid = np.where(
                    inb, (nb[:, 0] * G2 + nb[:, 1]) * G2 + nb[:, 2], 0)
                s0 = np.where(inb, start[nbid], 0)
                e0 = np.where(inb, end[nbid], 0)
                for r in range(CAP):
                    idx = s0 + r
                    good = idx < e0
                    pidx = sortp[np.where(good, idx, 0)]
                    d2 = ((qp.astype(np.float64)
                           - xp[pidx].astype(np.float64)) ** 2).sum(1)
                    cols.append(np.where(good, d2, np.inf))
    dmat = np.stack(cols, axis=1)                      # (Q, 27*CAP)
    d10 = np.sqrt(np.partition(dmat, 9, axis=1)[:, 9])
    got10 = np.isfinite(d10)
    b_q = np.where(got10, np.minimum(b_q, d10 + 1e-3), b_q)

    # sort queries by cell id -> spatially tight blocks
    qcell = (qc[:, 0] * G + qc[:, 1]) * G + qc[:, 2]
    order = np.argsort(qcell, kind="stable")

    # candidate mask per block: union of sub-cluster bboxes (blocks that
    # straddle a sort-wrap boundary would otherwise span half the cube)
    def region_mask(qs, depth):
        lo = (qp[qs] - b_q[qs][:, None]).min(axis=0)
        hi = (qp[qs] + b_q[qs][:, None]).max(axis=0)
        span = hi - lo
        if depth == 0 or span.max() < 0.3 or len(qs) < 8:
            return ((xp >= lo) & (xp <= hi)).all(axis=1)
        ax = int(np.argmax(span))
        med = np.median(qp[qs, ax])
        sel = qp[qs, ax] <= med
        if sel.all() or not sel.any():
            return ((xp >= lo) & (xp <= hi)).all(axis=1)
        return (region_mask(qs[sel], depth - 1)
                | region_mask(qs[~sel], depth - 1))

    blk_gids = []
    for gb in range(GBLK):
        qs = order[gb * P:(gb + 1) * P]
        m = region_mask(qs, 3)
        blk_gids.append(np.where(m)[0].astype(np.int32))
    return order, blk_gids


def _prep_aug(xp, qp):
    f = np.float32
    px, py, pz = xp[:, 0], xp[:, 1], xp[:, 2]
    p2 = (px * px + py * py) + pz * pz
    pa = np.stack([np.ones(len(xp), f), p2, px, py, pz]).astype(f)
    qx, qy, qz = qp[:, 0], qp[:, 1], qp[:, 2]
    q2 = (qx * qx + qy * qy) + qz * qz
    qa = np.stack([q2, np.ones(len(qp), f), -2 * qx, -2 * qy, -2 * qz])
    return pa, qa.astype(f)


def kernel(x, p_grid):
    from concourse.bass_utils import run_bass_kernel_spmd

    f = np.float32
    xp = np.asarray(x, f).reshape(N, 3)
    qp = np.asarray(p_grid, f).reshape(Q, 3)

    order, blk_gids = _host_prune(xp, qp)
    pa, qa_full = _prep_aug(xp, qp)

    # Deal blocks round-robin by descending candidate count: slot j of core c
    # gets global block rank[j*8+c], so slot j has the same (max-of-8) width
    # on every core and the one NEFF serves all cores.
    counts = np.array([len(g) for g in blk_gids])
    rank = np.argsort(-counts, kind="stable")
    slot_blk = rank.reshape(NBLK, N_CORES)                 # [slot, core]
    npbs = [int(((counts[slot_blk[j]].max() + 127) // 128) * 128)
            for j in range(NBLK)]

    if ("nc", tuple(npbs)) not in _CACHE:
        _CACHE[("nc", tuple(npbs))] = _build_bass(npbs)
    nc = _CACHE[("nc", tuple(npbs))]

    sent = np.array([1.0, 3 * SENT * SENT, SENT, SENT, SENT], f)
    in_maps = []
    final_order = np.empty(Q, np.int64)
    for c in range(N_CORES):
        parts = []
        qidx = []
        for j in range(NBLK):
            g = blk_gids[slot_blk[j, c]]
            pad = np.empty((K, npbs[j]), f)
            pad[:] = sent[:, None]
            pad[:, :len(g)] = pa[:, g]
            parts.append(pad)
            qidx.append(order[slot_blk[j, c] * P:(slot_blk[j, c] + 1) * P])
        qidx = np.concatenate(qidx)
        final_order[c * QL:(c + 1) * QL] = qidx
        parts.append(qa_full[:, qidx])
        in_maps.append({"pq": np.ascontiguousarray(
            np.concatenate(parts, axis=1))})

    res = run_bass_kernel_spmd(nc, in_maps, core_ids=list(range(N_CORES)),
                               **_CACHE.get("run_kwargs", {}))
    _CACHE["last_results"] = res
    iv = np.stack([res.results[c]["out_iv"] for c in range(N_CORES)])
    iv = iv.reshape(Q, 32)
    lidx = iv[:, :10].astype(np.int64)                    # local idx, rank order
    vals = iv[:, 16:26]

    # local -> global ids via per-slot tables (padded to the widest slot)
    width = max(npbs)
    gid_tab = np.zeros((Q // P, width), np.int32)
    row = 0
    for c in range(N_CORES):
        for j in range(NBLK):
            g = blk_gids[slot_blk[j, c]]
            gid_tab[row, :len(g)] = g
            row += 1
    blk_of_q = np.repeat(np.arange(Q // P), P)
    gidx = gid_tab[blk_of_q[:, None], lidx]
    valid = vals >= np.float32(-RADIUS2)
    mapping_s = np.where(valid, gidx, 0).astype(np.int32)
    outputs_s = np.where(valid[..., None], xp[mapping_s], np.float32(0.0))

    # inverse permutation back to original query order
    inv = np.empty(Q, np.int64)
    inv[final_order] = np.arange(Q)
    mapping = mapping_s[inv]
    outputs = outputs_s[inv]
    return mapping[None], outputs[None].astype(np.float32)


# revision 37
# speedup vs baseline: 3.8163x; 1.0385x over previous
"""Ball-query KNN (radius=0.25, k=10) for Q=16384 queries over N=16384 points.

Strategy (8 NeuronCores, queries sharded 2048/core, spatially pruned):
  - Host: bucket points on a 32^3 grid; per query, find a PROVABLY safe
    upper bound b_q on its 10th-NN distance (smallest cell ring whose cube
    holds >= 10 points => b_q = (k+1)*h*sqrt(3)).  Sort queries by cell so
    each 128-query block is spatially tight; the block's candidate list is
    every point within the block bbox expanded per-query by b_q.  Blocks are
    dealt round-robin by descending candidate count so block-slot j has the
    same width on every core (one SPMD NEFF, per-slot widths, sentinel pad).
  - PE: augmented K=5 fp32 matmul computes d2 = q2 + p2 - 2 q.p per block
    over its NPB candidates (lhsT rows per query: [q2, 1, -2qx, -2qy,
    -2qz]; rhs rows per candidate point: [1, p2, px, py, pz]).
  - ACT: PSUM -> SBUF copy with scale=-1 => s = -d2.
  - DVE: exact fp32 top-16 of each row via max8 + max_index +
    match_replace + max8 + max_index (order and ties exactly as the
    value-sorted reference).
  - Host: local->global index decode, validity (s >= -r^2), mapping
    zeroing, coords gather, inverse query permutation, unshard.
The candidate sets provably contain each query's true top-10, so the
result is identical to brute force over all 16384 points.
"""

import numpy as np

N_CORES = 8
N = 16384          # points
Q = 16384          # queries (flattened 32*32*16)
QL = Q // N_CORES  # queries per core
P = 128            # partitions / queries per block
NBLK = QL // P     # 16 blocks per core
GBLK = Q // P      # 128 global blocks
K = 5              # augmented contraction dim
MM = 512           # matmul free dim (1 bank fp32)
CH = 2048          # psum chunk (4 banks)
RADIUS2 = 0.0625   # 0.25**2
NEG_INF = -3.0e38
G = 32             # host pruning grid
SENT = 4.0         # sentinel coordinate (far outside the unit cube)

_CACHE = {}


def _build_bass(npbs):
    import concourse.bacc as bacc
    import concourse.mybir as mybir
    from concourse.tile import TileContext

    f32 = mybir.dt.float32
    u32 = mybir.dt.uint32

    nc = bacc.Bacc("TRN2", target_bir_lowering=False, debug=False,
                   enable_asserts=False)
    # per-slot candidate points (augmented, variable widths) then the core's
    # queries (augmented) in ONE tensor: a single DMA = single semaphore, so
    # the first matmul carries one sync wait (walrus limit on LDWEIGHTS).
    tot = sum(npbs)
    offs = np.concatenate([[0], np.cumsum(npbs)]).tolist()
    pq_d = nc.dram_tensor("pq", [K, tot + QL], f32,
                          kind="ExternalInput").ap()
    out_d = nc.dram_tensor("out_iv", [QL, 32], f32, kind="ExternalOutput").ap()

    with TileContext(nc) as tc:
        with (
            tc.tile_pool(name="consts", bufs=1) as constp,
            tc.tile_pool(name="pablk", bufs=3) as pablkp,
            tc.tile_pool(name="ps", bufs=2, space="PSUM") as psp,
            tc.tile_pool(name="sblk", bufs=2) as sblkp,
            tc.tile_pool(name="small", bufs=4) as smallp,
            tc.tile_pool(name="outb", bufs=3) as outbp,
        ):
            qa_all = constp.tile([K, QL], f32, tag="qa")
            nc.sync.dma_start(out=qa_all, in_=pq_d[:, tot:])

            for b in range(NBLK):
                npb = npbs[b]
                qa_sb = qa_all[:, b * P:(b + 1) * P]
                pa_sb = pablkp.tile([K, npb], f32, tag="pablk")
                nc.sync.dma_start(out=pa_sb,
                                  in_=pq_d[:, offs[b]:offs[b + 1]])

                s_blk = sblkp.tile([P, npb], f32, tag="s_blk")
                for c in range(0, npb, CH):
                    ch = min(CH, npb - c)
                    ps = psp.tile([P, CH], f32, tag="ps")
                    for m in range(0, ch, MM):
                        mm = min(MM, ch - m)
                        nc.tensor.matmul(
                            ps[:, m:m + mm],
                            lhsT=qa_sb,
                            rhs=pa_sb[:, c + m:c + m + mm],
                            start=True, stop=True,
                        )
                        nc.scalar.activation(
                            s_blk[:, c + m:c + m + mm],
                            ps[:, m:m + mm],
                            mybir.ActivationFunctionType.Copy,
                            scale=-1.0,
                        )

                out_sb = outbp.tile([P, 32], f32, tag="out_sb")

                # exact fp32 top-16 (values + local indices), rank order
                t8v = smallp.tile([P, 8], f32, tag="t8v")
                nc.vector.max(out=t8v, in_=s_blk)
                t8p = smallp.tile([P, 8], u32, tag="t8p")
                nc.vector.max_index(out=t8p, in_max=t8v, in_values=s_blk)

                s_blk2 = sblkp.tile([P, npb], f32, tag="s_blk2")
                nc.vector.match_replace(out=s_blk2, in_to_replace=t8v,
                                        in_values=s_blk, imm_value=NEG_INF)
                n8v = smallp.tile([P, 8], f32, tag="n8v")
                nc.vector.max(out=n8v, in_=s_blk2)
                n8p = smallp.tile([P, 8], u32, tag="n8p")
                nc.vector.max_index(out=n8p, in_max=n8v, in_values=s_blk2)

                nc.vector.tensor_copy(out_sb[:, 0:8], t8p)
                nc.vector.tensor_copy(out_sb[:, 8:16], n8p)
                nc.vector.tensor_copy(out_sb[:, 16:24], t8v)
                nc.vector.tensor_copy(out_sb[:, 24:32], n8v)

                nc.sync.dma_start(out=out_d[b * P:(b + 1) * P, :], in_=out_sb)

    nc.finalize()
    return nc


def _host_prune(xp, qp):
    """Spatially sort queries and build per-block candidate lists.

    Returns (order, blk_gids [GBLK, npb] int32, npb).  Candidate list of
    each block provably contains every member query's true 10 nearest
    neighbors (by the ring-count bound).
    """
    h = 1.0 / G
    pc = np.clip((xp * G).astype(np.int64), 0, G - 1)       # (N,3)
    qc = np.clip((qp * G).astype(np.int64), 0, G - 1)       # (Q,3)
    # 3D cell counts and inclusive prefix sums for O(1) cube counts
    cnt = np.zeros((G, G, G), np.int64)
    np.add.at(cnt, (pc[:, 0], pc[:, 1], pc[:, 2]), 1)
    S = np.zeros((G + 1, G + 1, G + 1), np.int64)
    S[1:, 1:, 1:] = cnt.cumsum(0).cumsum(1).cumsum(2)

    def cube_count(c, k):
        lo = np.clip(c - k, 0, G)
        hi = np.clip(c + k + 1, 0, G)
        return (S[hi[:, 0], hi[:, 1], hi[:, 2]]
                - S[lo[:, 0], hi[:, 1], hi[:, 2]]
                - S[hi[:, 0], lo[:, 1], hi[:, 2]]
                - S[hi[:, 0], hi[:, 1], lo[:, 2]]
                + S[lo[:, 0], lo[:, 1], hi[:, 2]]
                + S[lo[:, 0], hi[:, 1], lo[:, 2]]
                + S[hi[:, 0], lo[:, 1], lo[:, 2]]
                - S[lo[:, 0], lo[:, 1], lo[:, 2]])

    b_q = np.full(Q, -1.0)
    for k in range(1, 8):
        need = b_q < 0
        if not need.any():
            break
        cc = cube_count(qc[need], k)
        ok = cc >= 10
        idxs = np.where(need)[0][ok]
        b_q[idxs] = (k + 1) * h * np.sqrt(3.0)
    assert (b_q > 0).all(), "ring bound not found for some query"

    # tighten: 10th-smallest exact distance to a capped local subset is a
    # valid upper bound on the 10th-NN distance (any subset works).  Gather
    # up to CAP points per cell from the 27-cell cube on a coarse grid.
    G2 = 16
    CAP = 8
    pc2 = np.clip((xp * G2).astype(np.int64), 0, G2 - 1)
    qc2 = np.clip((qp * G2).astype(np.int64), 0, G2 - 1)
    cell2 = (pc2[:, 0] * G2 + pc2[:, 1]) * G2 + pc2[:, 2]
    sortp = np.argsort(cell2, kind="stable")
    cs = cell2[sortp]
    start = np.searchsorted(cs, np.arange(G2 ** 3))
    end = np.searchsorted(cs, np.arange(G2 ** 3), side="right")
    cols = []
    for dx in (-1, 0, 1):
        for dy in (-1, 0, 1):
            for dz in (-1, 0, 1):
                nb = qc2 + np.array([dx, dy, dz])
                inb = ((nb >= 0) & (nb < G2)).all(axis=1)
                nbid = np.where(
                    inb, (nb[:, 0] * G2 + nb[:, 1]) * G2 + nb[:, 2], 0)
                s0 = np.where(inb, start[nbid], 0)
                e0 = np.where(inb, end[nbid], 0)
                for r in range(CAP):
                    idx = s0 + r
                    good = idx < e0
                    pidx = sortp[np.where(good, idx, 0)]
                    d2 = ((qp.astype(np.float64)
                           - xp[pidx].astype(np.float64)) ** 2).sum(1)
                    cols.append(np.where(good, d2, np.inf))
    dmat = np.stack(cols, axis=1)                      # (Q, 27*CAP)
    d10 = np.sqrt(np.partition(dmat, 9, axis=1)[:, 9])
    got10 = np.isfinite(d10)
    b_q = np.where(got10, np.minimum(b_q, d10 + 1e-3), b_q)

    # sort queries by cell id -> spatially tight blocks
    qcell = (qc[:, 0] * G + qc[:, 1]) * G + qc[:, 2]
    order = np.argsort(qcell, kind="stable")

    # candidate mask per block: union of sub-cluster bboxes (blocks that
    # straddle a sort-wrap boundary would otherwise span half the cube)
    def region_mask(qs, depth):
        lo = (qp[qs] - b_q[qs][:, None]).min(axis=0)
        hi = (qp[qs] + b_q[qs][:, None]).max(axis=0)
        span = hi - lo
        if depth == 0 or span.max() < 0.3 or len(qs) < 8:
            return ((xp >= lo) & (xp <= hi)).all(axis=1)
        ax = int(np.argmax(span))
        med = np.median(qp[qs, ax])
        sel = qp[qs, ax] <= med
        if sel.all() or not sel.any():
            return ((xp >= lo) & (xp <= hi)).all(axis=1)
        return (region_mask(qs[sel], depth - 1)
                | region_mask(qs[~sel], depth - 1))

    blk_gids = []
    for gb in range(GBLK):
        qs = order[gb * P:(gb + 1) * P]
        m = region_mask(qs, 3)
        blk_gids.append(np.where(m)[0].astype(np.int32))
    return order, blk_gids


def _prep_aug(xp, qp):
    f = np.float32
    px, py, pz = xp[:, 0], xp[:, 1], xp[:, 2]
    p2 = (px * px + py * py) + pz * pz
    pa = np.stack([np.ones(len(xp), f), p2, px, py, pz]).astype(f)
    qx, qy, qz = qp[:, 0], qp[:, 1], qp[:, 2]
    q2 = (qx * qx + qy * qy) + qz * qz
    qa = np.stack([q2, np.ones(len(qp), f), -2 * qx, -2 * qy, -2 * qz])
    return pa, qa.astype(f)


def kernel(x, p_grid):
    from concourse.bass_utils import run_bass_kernel_spmd

    f = np.float32
    xp = np.asarray(x, f).reshape(N, 3)
    qp = np.asarray(p_grid, f).reshape(Q, 3)

    order, blk_gids = _host_prune(xp, qp)
    pa, qa_full = _prep_aug(xp, qp)

    # Deal blocks round-robin by descending candidate count: slot j of core c
    # gets global block rank[j*8+c], so slot j has the same (max-of-8) width
    # on every core and the one NEFF serves all cores.
    counts = np.array([len(g) for g in blk_gids])
    rank = np.argsort(-counts, kind="stable")
    slot_blk = rank.reshape(NBLK, N_CORES)                 # [slot, core]
    npbs = [int(((counts[slot_blk[j]].max() + 63) // 64) * 64)
            for j in range(NBLK)]

    if ("nc", tuple(npbs)) not in _CACHE:
        _CACHE[("nc", tuple(npbs))] = _build_bass(npbs)
    nc = _CACHE[("nc", tuple(npbs))]

    sent = np.array([1.0, 3 * SENT * SENT, SENT, SENT, SENT], f)
    in_maps = []
    final_order = np.empty(Q, np.int64)
    for c in range(N_CORES):
        parts = []
        qidx = []
        for j in range(NBLK):
            g = blk_gids[slot_blk[j, c]]
            pad = np.empty((K, npbs[j]), f)
            pad[:] = sent[:, None]
            pad[:, :len(g)] = pa[:, g]
            parts.append(pad)
            qidx.append(order[slot_blk[j, c] * P:(slot_blk[j, c] + 1) * P])
        qidx = np.concatenate(qidx)
        final_order[c * QL:(c + 1) * QL] = qidx
        parts.append(qa_full[:, qidx])
        in_maps.append({"pq": np.ascontiguousarray(
            np.concatenate(parts, axis=1))})

    res = run_bass_kernel_spmd(nc, in_maps, core_ids=list(range(N_CORES)),
                               **_CACHE.get("run_kwargs", {}))
    _CACHE["last_results"] = res
    iv = np.stack([res.results[c]["out_iv"] for c in range(N_CORES)])
    iv = iv.reshape(Q, 32)
    lidx = iv[:, :10].astype(np.int64)                    # local idx, rank order
    vals = iv[:, 16:26]

    # local -> global ids via per-slot tables (padded to the widest slot)
    width = max(npbs)
    gid_tab = np.zeros((Q // P, width), np.int32)
    row = 0
    for c in range(N_CORES):
        for j in range(NBLK):
            g = blk_gids[slot_blk[j, c]]
            gid_tab[row, :len(g)] = g
            row += 1
    blk_of_q = np.repeat(np.arange(Q // P), P)
    gidx = gid_tab[blk_of_q[:, None], lidx]
    valid = vals >= np.float32(-RADIUS2)
    mapping_s = np.where(valid, gidx, 0).astype(np.int32)
    outputs_s = np.where(valid[..., None], xp[mapping_s], np.float32(0.0))

    # inverse permutation back to original query order
    inv = np.empty(Q, np.int64)
    inv[final_order] = np.arange(Q)
    mapping = mapping_s[inv]
    outputs = outputs_s[inv]
    return mapping[None], outputs[None].astype(np.float32)


# revision 38
# speedup vs baseline: 3.9155x; 1.0260x over previous
"""Ball-query KNN (radius=0.25, k=10) for Q=16384 queries over N=16384 points.

Strategy (8 NeuronCores, queries sharded 2048/core, spatially pruned):
  - Host: bucket points on a 32^3 grid; per query, find a PROVABLY safe
    upper bound b_q on its 10th-NN distance (smallest cell ring whose cube
    holds >= 10 points => b_q = (k+1)*h*sqrt(3)).  Sort queries by cell so
    each 128-query block is spatially tight; the block's candidate list is
    every point within the block bbox expanded per-query by b_q.  Blocks are
    dealt round-robin by descending candidate count so block-slot j has the
    same width on every core (one SPMD NEFF, per-slot widths, sentinel pad).
  - PE: augmented K=5 fp32 matmul computes d2 = q2 + p2 - 2 q.p per block
    over its NPB candidates (lhsT rows per query: [q2, 1, -2qx, -2qy,
    -2qz]; rhs rows per candidate point: [1, p2, px, py, pz]).
  - ACT: PSUM -> SBUF copy with scale=-1 => s = -d2.
  - DVE: exact fp32 top-16 of each row via max8 + max_index +
    match_replace + max8 + max_index (order and ties exactly as the
    value-sorted reference).
  - Host: local->global index decode, validity (s >= -r^2), mapping
    zeroing, coords gather, inverse query permutation, unshard.
The candidate sets provably contain each query's true top-10, so the
result is identical to brute force over all 16384 points.
"""

import numpy as np

N_CORES = 8
N = 16384          # points
Q = 16384          # queries (flattened 32*32*16)
QL = Q // N_CORES  # queries per core
P = 128            # partitions / queries per block
NBLK = QL // P     # 16 blocks per core
GBLK = Q // P      # 128 global blocks
K = 5              # augmented contraction dim
MM = 512           # matmul free dim (1 bank fp32)
CH = 2048          # psum chunk (4 banks)
RADIUS2 = 0.0625   # 0.25**2
NEG_INF = -3.0e38
G = 32             # host pruning grid
SENT = 4.0         # sentinel coordinate (far outside the unit cube)

_CACHE = {}


def _build_bass(npbs):
    import concourse.bacc as bacc
    import concourse.mybir as mybir
    from concourse.tile import TileContext

    f32 = mybir.dt.float32
    u32 = mybir.dt.uint32

    nc = bacc.Bacc("TRN2", target_bir_lowering=False, debug=False,
                   enable_asserts=False)
    # per-slot candidate points (augmented, variable widths) then the core's
    # queries (augmented) in ONE tensor: a single DMA = single semaphore, so
    # the first matmul carries one sync wait (walrus limit on LDWEIGHTS).
    tot = sum(npbs)
    offs = np.concatenate([[0], np.cumsum(npbs)]).tolist()
    pq_d = nc.dram_tensor("pq", [K, tot + QL], f32,
                          kind="ExternalInput").ap()
    out_d = nc.dram_tensor("out_iv", [QL, 32], f32, kind="ExternalOutput").ap()

    with TileContext(nc) as tc:
        with (
            tc.tile_pool(name="consts", bufs=1) as constp,
            tc.tile_pool(name="pablk", bufs=3) as pablkp,
            tc.tile_pool(name="ps", bufs=2, space="PSUM") as psp,
            tc.tile_pool(name="sblk", bufs=2) as sblkp,
            tc.tile_pool(name="small", bufs=4) as smallp,
            tc.tile_pool(name="outb", bufs=3) as outbp,
        ):
            qa_all = constp.tile([K, QL], f32, tag="qa")
            nc.sync.dma_start(out=qa_all, in_=pq_d[:, tot:])

            for b in range(NBLK):
                npb = npbs[b]
                qa_sb = qa_all[:, b * P:(b + 1) * P]
                pa_sb = pablkp.tile([K, npb], f32, tag="pablk")
                nc.sync.dma_start(out=pa_sb,
                                  in_=pq_d[:, offs[b]:offs[b + 1]])

                s_blk = sblkp.tile([P, npb], f32, tag="s_blk")
                for c in range(0, npb, CH):
                    ch = min(CH, npb - c)
                    ps = psp.tile([P, CH], f32, tag="ps")
                    for m in range(0, ch, MM):
                        mm = min(MM, ch - m)
                        nc.tensor.matmul(
                            ps[:, m:m + mm],
                            lhsT=qa_sb,
                            rhs=pa_sb[:, c + m:c + m + mm],
                            start=True, stop=True,
                        )
                        nc.scalar.activation(
                            s_blk[:, c + m:c + m + mm],
                            ps[:, m:m + mm],
                            mybir.ActivationFunctionType.Copy,
                            scale=-1.0,
                        )

                out_sb = outbp.tile([P, 32], f32, tag="out_sb")

                # exact fp32 top-16 (values + local indices), rank order;
                # max8 writes values straight into the output tile
                t8v = out_sb[:, 16:24]
                n8v = out_sb[:, 24:32]
                nc.vector.max(out=t8v, in_=s_blk)
                t8p = smallp.tile([P, 8], u32, tag="t8p")
                nc.vector.max_index(out=t8p, in_max=t8v, in_values=s_blk)

                s_blk2 = sblkp.tile([P, npb], f32, tag="s_blk2")
                nc.vector.match_replace(out=s_blk2, in_to_replace=t8v,
                                        in_values=s_blk, imm_value=NEG_INF)
                nc.vector.max(out=n8v, in_=s_blk2)
                n8p = smallp.tile([P, 8], u32, tag="n8p")
                nc.vector.max_index(out=n8p, in_max=n8v, in_values=s_blk2)

                nc.vector.tensor_copy(out_sb[:, 0:8], t8p)
                nc.vector.tensor_copy(out_sb[:, 8:16], n8p)

                nc.sync.dma_start(out=out_d[b * P:(b + 1) * P, :], in_=out_sb)

    nc.finalize()
    return nc


def _host_prune(xp, qp):
    """Spatially sort queries and build per-block candidate lists.

    Returns (order, blk_gids [GBLK, npb] int32, npb).  Candidate list of
    each block provably contains every member query's true 10 nearest
    neighbors (by the ring-count bound).
    """
    h = 1.0 / G
    pc = np.clip((xp * G).astype(np.int64), 0, G - 1)       # (N,3)
    qc = np.clip((qp * G).astype(np.int64), 0, G - 1)       # (Q,3)
    # 3D cell counts and inclusive prefix sums for O(1) cube counts
    cnt = np.zeros((G, G, G), np.int64)
    np.add.at(cnt, (pc[:, 0], pc[:, 1], pc[:, 2]), 1)
    S = np.zeros((G + 1, G + 1, G + 1), np.int64)
    S[1:, 1:, 1:] = cnt.cumsum(0).cumsum(1).cumsum(2)

    def cube_count(c, k):
        lo = np.clip(c - k, 0, G)
        hi = np.clip(c + k + 1, 0, G)
        return (S[hi[:, 0], hi[:, 1], hi[:, 2]]
                - S[lo[:, 0], hi[:, 1], hi[:, 2]]
                - S[hi[:, 0], lo[:, 1], hi[:, 2]]
                - S[hi[:, 0], hi[:, 1], lo[:, 2]]
                + S[lo[:, 0], lo[:, 1], hi[:, 2]]
                + S[lo[:, 0], hi[:, 1], lo[:, 2]]
                + S[hi[:, 0], lo[:, 1], lo[:, 2]]
                - S[lo[:, 0], lo[:, 1], lo[:, 2]])

    b_q = np.full(Q, -1.0)
    for k in range(1, 8):
        need = b_q < 0
        if not need.any():
            break
        cc = cube_count(qc[need], k)
        ok = cc >= 10
        idxs = np.where(need)[0][ok]
        b_q[idxs] = (k + 1) * h * np.sqrt(3.0)
    assert (b_q > 0).all(), "ring bound not found for some query"

    # tighten: 10th-smallest exact distance to a capped local subset is a
    # valid upper bound on the 10th-NN distance (any subset works).  Gather
    # up to CAP points per cell from the 27-cell cube on a coarse grid.
    G2 = 16
    CAP = 8
    pc2 = np.clip((xp * G2).astype(np.int64), 0, G2 - 1)
    qc2 = np.clip((qp * G2).astype(np.int64), 0, G2 - 1)
    cell2 = (pc2[:, 0] * G2 + pc2[:, 1]) * G2 + pc2[:, 2]
    sortp = np.argsort(cell2, kind="stable")
    cs = cell2[sortp]
    start = np.searchsorted(cs, np.arange(G2 ** 3))
    end = np.searchsorted(cs, np.arange(G2 ** 3), side="right")
    cols = []
    for dx in (-1, 0, 1):
        for dy in (-1, 0, 1):
            for dz in (-1, 0, 1):
                nb = qc2 + np.array([dx, dy, dz])
                inb = ((nb >= 0) & (nb < G2)).all(axis=1)
                nbid = np.where(
                    inb, (nb[:, 0] * G2 + nb[:, 1]) * G2 + nb[:, 2], 0)
                s0 = np.where(inb, start[nbid], 0)
                e0 = np.where(inb, end[nbid], 0)
                for r in range(CAP):
                    idx = s0 + r
                    good = idx < e0
                    pidx = sortp[np.where(good, idx, 0)]
                    d2 = ((qp.astype(np.float64)
                           - xp[pidx].astype(np.float64)) ** 2).sum(1)
                    cols.append(np.where(good, d2, np.inf))
    dmat = np.stack(cols, axis=1)                      # (Q, 27*CAP)
    d10 = np.sqrt(np.partition(dmat, 9, axis=1)[:, 9])
    got10 = np.isfinite(d10)
    b_q = np.where(got10, np.minimum(b_q, d10 + 1e-3), b_q)

    # sort queries by cell id -> spatially tight blocks
    qcell = (qc[:, 0] * G + qc[:, 1]) * G + qc[:, 2]
    order = np.argsort(qcell, kind="stable")

    # candidate mask per block: union of sub-cluster bboxes (blocks that
    # straddle a sort-wrap boundary would otherwise span half the cube)
    def region_mask(qs, depth):
        lo = (qp[qs] - b_q[qs][:, None]).min(axis=0)
        hi = (qp[qs] + b_q[qs][:, None]).max(axis=0)
        span = hi - lo
        if depth == 0 or span.max() < 0.3 or len(qs) < 8:
            return ((xp >= lo) & (xp <= hi)).all(axis=1)
        ax = int(np.argmax(span))
        med = np.median(qp[qs, ax])
        sel = qp[qs, ax] <= med
        if sel.all() or not sel.any():
            return ((xp >= lo) & (xp <= hi)).all(axis=1)
        return (region_mask(qs[sel], depth - 1)
                | region_mask(qs[~sel], depth - 1))

    blk_gids = []
    for gb in range(GBLK):
        qs = order[gb * P:(gb + 1) * P]
        m = region_mask(qs, 3)
        blk_gids.append(np.where(m)[0].astype(np.int32))
    return order, blk_gids


def _prep_aug(xp, qp):
    f = np.float32
    px, py, pz = xp[:, 0], xp[:, 1], xp[:, 2]
    p2 = (px * px + py * py) + pz * pz
    pa = np.stack([np.ones(len(xp), f), p2, px, py, pz]).astype(f)
    qx, qy, qz = qp[:, 0], qp[:, 1], qp[:, 2]
    q2 = (qx * qx + qy * qy) + qz * qz
    qa = np.stack([q2, np.ones(len(qp), f), -2 * qx, -2 * qy, -2 * qz])
    return pa, qa.astype(f)


def kernel(x, p_grid):
    from concourse.bass_utils import run_bass_kernel_spmd

    f = np.float32
    xp = np.asarray(x, f).reshape(N, 3)
    qp = np.asarray(p_grid, f).reshape(Q, 3)

    order, blk_gids = _host_prune(xp, qp)
    pa, qa_full = _prep_aug(xp, qp)

    # Deal blocks round-robin by descending candidate count: slot j of core c
    # gets global block rank[j*8+c], so slot j has the same (max-of-8) width
    # on every core and the one NEFF serves all cores.
    counts = np.array([len(g) for g in blk_gids])
    rank = np.argsort(-counts, kind="stable")
    slot_blk = rank.reshape(NBLK, N_CORES)                 # [slot, core]
    npbs = [int(((counts[slot_blk[j]].max() + 63) // 64) * 64)
            for j in range(NBLK)]

    if ("nc", tuple(npbs)) not in _CACHE:
        _CACHE[("nc", tuple(npbs))] = _build_bass(npbs)
    nc = _CACHE[("nc", tuple(npbs))]

    sent = np.array([1.0, 3 * SENT * SENT, SENT, SENT, SENT], f)
    in_maps = []
    final_order = np.empty(Q, np.int64)
    for c in range(N_CORES):
        parts = []
        qidx = []
        for j in range(NBLK):
            g = blk_gids[slot_blk[j, c]]
            pad = np.empty((K, npbs[j]), f)
            pad[:] = sent[:, None]
            pad[:, :len(g)] = pa[:, g]
            parts.append(pad)
            qidx.append(order[slot_blk[j, c] * P:(slot_blk[j, c] + 1) * P])
        qidx = np.concatenate(qidx)
        final_order[c * QL:(c + 1) * QL] = qidx
        parts.append(qa_full[:, qidx])
        in_maps.append({"pq": np.ascontiguousarray(
            np.concatenate(parts, axis=1))})

    res = run_bass_kernel_spmd(nc, in_maps, core_ids=list(range(N_CORES)),
                               **_CACHE.get("run_kwargs", {}))
    _CACHE["last_results"] = res
    iv = np.stack([res.results[c]["out_iv"] for c in range(N_CORES)])
    iv = iv.reshape(Q, 32)
    lidx = iv[:, :10].astype(np.int64)                    # local idx, rank order
    vals = iv[:, 16:26]

    # local -> global ids via per-slot tables (padded to the widest slot)
    width = max(npbs)
    gid_tab = np.zeros((Q // P, width), np.int32)
    row = 0
    for c in range(N_CORES):
        for j in range(NBLK):
            g = blk_gids[slot_blk[j, c]]
            gid_tab[row, :len(g)] = g
            row += 1
    blk_of_q = np.repeat(np.arange(Q // P), P)
    gidx = gid_tab[blk_of_q[:, None], lidx]
    valid = vals >= np.float32(-RADIUS2)
    mapping_s = np.where(valid, gidx, 0).astype(np.int32)
    outputs_s = np.where(valid[..., None], xp[mapping_s], np.float32(0.0))

    # inverse permutation back to original query order
    inv = np.empty(Q, np.int64)
    inv[final_order] = np.arange(Q)
    mapping = mapping_s[inv]
    outputs = outputs_s[inv]
    return mapping[None], outputs[None].astype(np.float32)
